# revision 1
# baseline (speedup 1.0000x reference)
"""CFConv kernel v2 — For_i-looped, static-instruction-lean.

Cost law on this axon-tunneled TRN2: each STATIC NEFF instruction costs
~70us fixed + ~0.2-0.5us/column; For_i loop re-executions are ~free;
multi-engine back-edges ~350us. So: put every fat op inside one For_i
loop over 2048-column slices of the merged pair stream (2 batches x
16384 pairs = 32768 columns per core).

Per-core dataflow (F=128 on partitions, r2 = b*16384 + a*64 + n):
  pre:  weights/biases, fijT [64,32768] bf16, xt [128,512], rm [128,512],
        idx [128,2048] i16, ytab = win.T @ xt -> [128,512] f32,
        s_pair = 0.5*(cos(pi r/5)+1)*(r<5)*mask -> srow [1,32768] bf16
  loop i in 0..16 (W=2048):
        4x mm1 -> psA; Exp; Ln -> H bf16; 4x mm2 -> psB;
        gather ytab[idx] -> ynbh f32; bcast srow -> sb bf16;
        STT (psB+fb2)*ynbh -> t bf16; TT t*sb -> P bf16;
        reduce P[128,32,64] -> Qs[:, 32i:32i+32] f32
  post: Qs->bf16; psO = wout.T @ Qs; Exp; Ln; DMA out [o, a2]
"""
import math
import numpy as np
import concourse.bass as bass
import concourse.bacc as bacc
import concourse.mybir as mybir
from concourse import tile
from concourse import bass2jax
from concourse.bass import ds, ts

f32 = mybir.dt.float32
bf16 = mybir.dt.bfloat16
i16 = mybir.dt.int16
i32 = mybir.dt.int32
AF = mybir.ActivationFunctionType
ALU = mybir.AluOpType

B, A, N, G, F = 16, 256, 64, 64, 128
R = A * N
N_CORES = 8
BPC = B // N_CORES
R2 = BPC * R          # 32768 pair-columns per core
A2 = BPC * A          # 512 atoms per core
CUTOFF = 5.0


def host_prep(inputs, n_cores=N_CORES):
    f_ij = np.asarray(inputs["f_ij"], np.float32)
    x = np.asarray(inputs["x"], np.float32)
    r_ij = np.asarray(inputs["r_ij"], np.float32)
    mask = np.asarray(inputs["pairwise_mask"], np.float32)
    nbr = np.asarray(inputs["neighbors"]).astype(np.int64)
    w = lambda k: np.ascontiguousarray(np.asarray(inputs[k], np.float32))
    b3 = np.stack([np.asarray(inputs["fb1"], np.float32),
                   np.asarray(inputs["fb2"], np.float32),
                   np.asarray(inputs["b_f2out"], np.float32)], axis=1)
    in_maps = []
    for c in range(n_cores):
        sl = slice(c * BPC, (c + 1) * BPC)
        fijT = np.ascontiguousarray(
            f_ij[sl].reshape(R2, G).T)                       # [64, 32768]
        xt = np.ascontiguousarray(x[sl].reshape(A2, F).T)    # [128, 512]
        rmr = r_ij[sl].reshape(128, R2 // 128)
        rmm = mask[sl].reshape(128, R2 // 128)
        rm = np.ascontiguousarray(np.concatenate([rmr, rmm], axis=1))
        flat = nbr[sl].reshape(BPC, R) + (np.arange(BPC) * A)[:, None]
        wrapped = flat.reshape(R2 // 16, 16).T.astype(np.int16)  # [16, 2048]
        idx = np.ascontiguousarray(np.tile(wrapped, (8, 1)))     # [128, 2048]
        in_maps.append({
            "fijT": fijT, "xt": xt, "rm": rm, "idx": idx,
            "fw1": w("fw1"), "fw2": w("fw2"), "w_in2f": w("w_in2f"),
            "w_f2out": w("w_f2out"), "b3": np.ascontiguousarray(b3),
        })
    return in_maps


def build_nc(bpc=BPC, num_devices=N_CORES, reps=1, W=2048):
    NI = R2 // W          # loop iterations
    SUB = W // 512        # matmuls per GEMM per iteration
    AW = W // N           # atoms per iteration
    nc = bacc.Bacc("TRN2", target_bir_lowering=False, debug=False,
                   num_devices=num_devices)
    D = nc.declare_dram_parameter
    fijT_d = D("fijT", [G, R2], f32, isOutput=False)
    xt_d = D("xt", [F, A2], f32, isOutput=False)
    rm_d = D("rm", [128, 2 * (R2 // 128)], f32, isOutput=False)
    idx_d = D("idx", [128, R2 // 16], i16, isOutput=False)
    fw1_d = D("fw1", [G, F], f32, isOutput=False)
    fw2_d = D("fw2", [F, F], f32, isOutput=False)
    win_d = D("w_in2f", [F, F], f32, isOutput=False)
    wout_d = D("w_f2out", [F, F], f32, isOutput=False)
    b3_d = D("b3", [F, 3], f32, isOutput=False)
    out_d = D("out", [F, A2], f32, isOutput=True)
    QC = R2 // 128        # rm column split (r | mask)

    with tile.TileContext(nc) as tc:
        with tc.tile_pool(name="const", bufs=1) as cpool, \
             tc.tile_pool(name="work", bufs=1) as wpool, \
             tc.tile_pool(name="psA", bufs=1, space="PSUM") as pa, \
             tc.tile_pool(name="psB", bufs=1, space="PSUM") as pb:

            fw1_sb = cpool.tile([G, F], bf16, tag="fw1")
            nc.gpsimd.dma_start(out=fw1_sb[:], in_=fw1_d[:, :])
            fw2_sb = cpool.tile([F, F], bf16, tag="fw2")
            nc.gpsimd.dma_start(out=fw2_sb[:], in_=fw2_d[:, :])
            win_sb = cpool.tile([F, F], bf16, tag="win")
            nc.gpsimd.dma_start(out=win_sb[:], in_=win_d[:, :])
            wout_sb = cpool.tile([F, F], bf16, tag="wout")
            nc.gpsimd.dma_start(out=wout_sb[:], in_=wout_d[:, :])
            b3_sb = cpool.tile([F, 3], f32, tag="b3")
            nc.sync.dma_start(out=b3_sb[:], in_=b3_d[:, :])
            half_sb = cpool.tile([128, 1], f32, tag="half")
            nc.gpsimd.memset(half_sb[:], 0.5)
            hpi_sb = cpool.tile([128, 1], f32, tag="hpi")
            nc.gpsimd.memset(hpi_sb[:], math.pi / 2.0)

            for rep in range(reps):
                fijT_sb = wpool.tile([G, R2], bf16, tag="fijT")
                nc.gpsimd.dma_start(out=fijT_sb[:], in_=fijT_d[:, :])
                xt_sb = wpool.tile([F, A2], bf16, tag="xt")
                nc.gpsimd.dma_start(out=xt_sb[:], in_=xt_d[:, :])
                rm_sb = wpool.tile([128, 2 * QC], f32, tag="rm")
                nc.sync.dma_start(out=rm_sb[:], in_=rm_d[:, :])
                idx_sb = wpool.tile([128, R2 // 16], i16, tag="idx")
                nc.sync.dma_start(out=idx_sb[:], in_=idx_d[:, :])

                psA = pa.tile([128, W], f32, tag="psA")
                psB = pb.tile([128, W], f32, tag="psB")

                # y table: [f, a2] = win.T @ xt
                nc.tensor.matmul(psA[:, :A2], win_sb[:], xt_sb[:],
                                 start=True, stop=True)
                ytab_sb = wpool.tile([128, A2], f32, tag="ytab")
                nc.vector.tensor_copy(ytab_sb[:], psA[:, :A2])

                # cutoff * mask -> s_pair bf16 [128, QC]
                c0_sb = wpool.tile([128, QC], f32, tag="c0")
                nc.scalar.activation(c0_sb[:], rm_sb[:, :QC], AF.Sin,
                                     bias=hpi_sb[:], scale=-math.pi / CUTOFF)
                cut_sb = wpool.tile([128, QC], f32, tag="cut")
                nc.vector.tensor_scalar(cut_sb[:], rm_sb[:, :QC], CUTOFF,
                                        None, ALU.is_lt)
                m2_sb = wpool.tile([128, QC], f32, tag="m2")
                nc.vector.scalar_tensor_tensor(
                    out=m2_sb[:], in0=cut_sb[:], scalar=0.5,
                    in1=rm_sb[:, QC:], op0=ALU.mult, op1=ALU.mult)
                sp_sb = wpool.tile([128, QC], bf16, tag="sp")
                nc.vector.scalar_tensor_tensor(
                    out=sp_sb[:], in0=c0_sb[:], scalar=1.0,
                    in1=m2_sb[:], op0=ALU.add, op1=ALU.mult)
                srow_sb = wpool.tile([1, R2], bf16, tag="srow")
                nc.sync.dma_start(
                    out=srow_sb.rearrange("o (p q) -> o p q", p=128),
                    in_=sp_sb[:])

                eH_sb = wpool.tile([128, W], f32, tag="eH")
                H_sb = wpool.tile([128, W], bf16, tag="H")
                ynbh_sb = wpool.tile([128, W], f32, tag="ynbh")
                sb_sb = wpool.tile([128, W], bf16, tag="sb")
                t_sb = wpool.tile([128, W], bf16, tag="t")
                P_sb = wpool.tile([128, W], bf16, tag="P")
                Qs_sb = wpool.tile([128, A2], f32, tag="Qs")

                with tc.For_i(0, NI, staggered_reset=True) as it:
                    for j in range(SUB):
                        nc.tensor.matmul(
                            psA[:, j * 512:(j + 1) * 512], fw1_sb[:],
                            fijT_sb[:, ds(it * W + j * 512, 512)],
                            start=True, stop=True)
                    nc.scalar.activation(eH_sb[:], psA[:], AF.Exp,
                                         bias=b3_sb[:, 0:1], scale=1.0)
                    nc.scalar.activation(H_sb[:], eH_sb[:], AF.Ln,
                                         bias=half_sb[:], scale=0.5)
                    for j in range(SUB):
                        nc.tensor.matmul(
                            psB[:, j * 512:(j + 1) * 512], fw2_sb[:],
                            H_sb[:, j * 512:(j + 1) * 512],
                            start=True, stop=True)
                    nc.gpsimd.ap_gather(
                        out_ap=ynbh_sb.unsqueeze(2),
                        in_ap=ytab_sb.unsqueeze(2),
                        idxs_ap=idx_sb[:, ds(it * (W // 16), W // 16)],
                        channels=128, num_elems=A2, d=1, num_idxs=W)
                    nc.gpsimd.partition_broadcast(
                        out_ap=sb_sb.bitcast(i32),
                        in_ap=srow_sb.bitcast(i32)[:, ds(it * (W // 2),
                                                         W // 2)],
                        channels=128)
                    nc.vector.scalar_tensor_tensor(
                        out=t_sb[:], in0=psB[:], scalar=b3_sb[:, 1:2],
                        in1=ynbh_sb[:], op0=ALU.add, op1=ALU.mult)
                    nc.vector.tensor_tensor(P_sb[:], t_sb[:], sb_sb[:],
                                            ALU.mult)
                    nc.vector.tensor_reduce(
                        Qs_sb[:, ds(it * AW, AW)],
                        P_sb.rearrange("p (a n) -> p a n", n=N),
                        mybir.AxisListType.X, ALU.add)

                Qb_sb = wpool.tile([128, A2], bf16, tag="Qb")
                nc.vector.tensor_copy(Qb_sb[:], Qs_sb[:])
                nc.tensor.matmul(psB[:, :A2], wout_sb[:], Qb_sb[:],
                                 start=True, stop=True)
                eo_sb = wpool.tile([128, A2], f32, tag="eo")
                nc.scalar.activation(eo_sb[:], psB[:, :A2], AF.Exp,
                                     bias=b3_sb[:, 2:3], scale=1.0)
                o_sb = wpool.tile([128, A2], f32, tag="o")
                nc.scalar.activation(o_sb[:], eo_sb[:], AF.Ln,
                                     bias=half_sb[:], scale=0.5)
                nc.sync.dma_start(out=out_d[:, :], in_=o_sb[:])
    nc.compile()
    return nc


_NC_CACHE = {}


def kernel(**inputs) -> np.ndarray:
    in_maps = host_prep(inputs)
    if "nc" not in _NC_CACHE:
        _NC_CACHE["nc"] = build_nc(bpc=BPC, num_devices=N_CORES, reps=1)
    nc = _NC_CACHE["nc"]
    results = bass2jax.run_bass_via_pjrt(nc, in_maps, n_cores=N_CORES)
    outs = []
    for r in results:
        o = np.asarray(r["out"], np.float32)        # [F, A2]
        outs.append(o.T.reshape(BPC, A, F))
    return np.concatenate(outs, axis=0)


_host_prep = host_prep

